# revision 9
# baseline (speedup 1.0000x reference)
"""Distributed Trainium2 (Bass/Tile) kernel for nn_DWAModel_64390149702175.

Distribution over 8 NeuronCores (core c, pair s = c%2, batch b = c//2):
 - embed/blockA/blockB/h_mid: batch b per core pair; within a pair, attention
   heads (4/4) and MLP DFF (1024/1024) are column/row-sharded with pair
   AllReduces for the two residual branches.
 - retrieval scan: pool row-sharded 8 ways; comb[b,n] = pool[n,:].(z@M) with
   M = sum_a wq_a wk_a^T/(A sqrt(DK)) (keys cache never materialized).
 - top-16: DVE max8/max_index two-level tournament, exact indices; gather of
   local pool rows via OOB-skipping indirect DMA; partial mix + AllReduce.
 - lm_head: vocab column-sharded 8 ways over allgathered h_out^T.
Block/lm_head matmuls run float32r (full-rate fp32, ~1e-4 rel err); the scan
runs plain fp32.
"""

import sys

sys.path.insert(0, "/opt/trn_rl_repo")

import numpy as np

import concourse.bacc as bacc
import concourse.bass as bass
import concourse.mybir as mybir
import concourse.tile as tile
from concourse.bass_utils import run_bass_kernel_spmd

F32 = mybir.dt.float32
F32R = mybir.dt.float32r
U32 = mybir.dt.uint32
AF = mybir.ActivationFunctionType
OP = mybir.AluOpType
AX = mybir.AxisListType

B, T, V = 4, 512, 32000
DA, DB, DFF, H = 512, 512, 2048, 8
N, R, A, DK, KMAX = 32768, 2, 4, 64, 16
D = R * (DA + DB)  # 2048
NCORES = 8
NSH = N // NCORES   # 4096 pool rows per core
VSH = V // NCORES   # 4000 vocab cols per core
HD = DA // H        # 64
HLOC = H // 2       # 4 heads per core
DH = HLOC * HD      # 256
FLOC = DFF // 2     # 1024
NT = T // 128       # 4 token tiles
EPS = 1e-5
PAIRS = [[0, 1], [2, 3], [4, 5], [6, 7]]
BGRPS = [[0, 2, 4, 6], [1, 3, 5, 7]]
ALLG = [list(range(NCORES))]


def _bld(nc, tc, REPS=1):
    P = 128
    _cms = {}

    def openpool(name, **kw):
        cm = tc.tile_pool(name=name, **kw)
        pool = cm.__enter__()
        _cms[id(pool)] = cm
        return pool

    def closepool(pool):
        _cms.pop(id(pool)).__exit__(None, None, None)

    def dram(name, shape, dtype=F32):
        return nc.declare_dram_parameter(name, list(shape), dtype, isOutput=False)

    # ---- inputs ----
    ids = dram("ids", [P, NT], U32)
    emb = dram("emb", [V, DA])
    posenc = dram("posenc", [T, DA])
    maskre = dram("maskre", [P, NT * T])
    ident_in = dram("ident", [P, P])
    pool_sh = dram("pool_sh", [NSH, D])
    poolT_sh = dram("poolT_sh", [D, NSH])
    lmh_sh = dram("lmh_sh", [DB, VSH], F32R)
    wqT = dram("wqT", [A, DK, DA], F32R)
    wkT = dram("wkT", [A, DK, D], F32R)
    w_base = dram("w_base", [DA, DB])
    eb = dram("eb", [B, 1])
    coff = dram("coff", [B, 1])
    segoff = dram("segoff", [32, 1])
    lam = dram("lam", [B, 1])
    warm = dram("warm", [B, 1])
    gam = dram("gam", [P, 1])
    reps = {}
    for nm in ["a_ln1g", "a_ln1b", "a_ln2g", "a_ln2b", "b_ln1g", "b_ln1b",
               "b_ln2g", "b_ln2b", "mid_g", "mid_b", "b_base"]:
        reps[nm] = dram(nm + "_rep", [P, DA])
    blkw = {}
    for pfx in ["a", "b"]:
        blkw[pfx] = dict(
            wqkv=dram(pfx + "_wqkv_sh", [DA, 3 * DH], F32R),   # q|k|v own heads
            wo=dram(pfx + "_wo_sh", [DH, DA], F32R),           # own head rows
            w1=dram(pfx + "_w1_sh", [DA, FLOC], F32R),
            w2=dram(pfx + "_w2_sh", [FLOC, DA], F32R),
        )

    logits = nc.declare_dram_parameter("logits", [B * T, VSH], F32, isOutput=True)

    # ---- global pools ----
    cst = openpool("cst", bufs=1)
    psA = openpool("psA", bufs=3, space="PSUM")
    psT = openpool("psT", bufs=3, space="PSUM")
    psS = openpool("psS", bufs=2, space="PSUM")
    dramp = openpool("dramp", bufs=1, space="DRAM")

    ident = cst.tile([P, P], F32, name="ident")
    nc.sync.dma_start(ident[:], ident_in[:])
    mask_sb = cst.tile([P, NT * T], F32, name="mask_sb")
    nc.sync.dma_start(mask_sb[:], maskre[:])
    rep_sb = {}
    for nm, dr in reps.items():
        t_ = cst.tile([P, DA], F32, name="rep_" + nm)
        nc.sync.dma_start(t_[:], dr[:])
        rep_sb[nm] = t_
    ids_sb = cst.tile([P, NT], U32, name="ids_sb")
    nc.sync.dma_start(ids_sb[:], ids[:])
    ones_col = cst.tile([P, 1], F32, name="ones_col")
    nc.vector.memset(ones_col[:], 1.0)
    small = {}
    for nm, dr, rows in [("eb", eb, B), ("coff", coff, B), ("segoff", segoff, 32),
                         ("lam", lam, B), ("warm", warm, B), ("gam", gam, P)]:
        t_ = cst.tile([rows, 1], F32, name="sml_" + nm)
        nc.sync.dma_start(t_[:], dr[:])
        small[nm] = t_
    wb_sb = []
    for dt_ in range(4):
        t_ = cst.tile([P, DB], F32, name=f"wb_sb{dt_}")
        nc.sync.dma_start(t_[:], w_base[dt_ * P:(dt_ + 1) * P, :])
        wb_sb.append(t_)

    # ------------------------------------------------ helpers
    def transpose_to(pool, src_ap, rows, cols, tag, bufs=5, out_dtype=F32R):
        pt = psT.tile([P, P], F32, tag="ptp", name="ptp")
        if src_ap.dtype == F32R:
            src_ap = src_ap.bitcast(F32)
        nc.tensor.transpose(pt[:cols, :rows], src_ap, ident[:rows, :rows])
        out = pool.tile([cols, rows], out_dtype, tag=tag, name=tag, bufs=bufs)
        nc.vector.tensor_copy(out[:], pt[:cols, :rows])
        return out

    def layernorm(pool, x_ap, g_rep, b_rep, nm):
        mu = pool.tile([P, 1], F32, tag="ln_mu", name="ln_mu", bufs=2)
        nc.vector.tensor_reduce(mu[:], x_ap, axis=AX.X, op=OP.add)
        nc.vector.tensor_scalar_mul(mu[:], mu[:], 1.0 / DA)
        xc = pool.tile([P, DA], F32, tag="ln_xc", name="ln_xc", bufs=2)
        nc.vector.tensor_scalar(xc[:], x_ap, mu[:], None, op0=OP.subtract)
        sq = pool.tile([P, DA], F32, tag="s512", name="ln_sq", bufs=3)
        var = pool.tile([P, 1], F32, tag="ln_var", name="ln_var", bufs=2)
        nc.scalar.activation(sq[:], xc[:], AF.Square, accum_out=var[:])
        rs = pool.tile([P, 1], F32, tag="ln_rs", name="ln_rs", bufs=2)
        nc.vector.tensor_scalar(rs[:], var[:], 1.0 / DA, EPS, op0=OP.mult, op1=OP.add)
        nc.scalar.activation(rs[:], rs[:], AF.Sqrt)
        nc.vector.reciprocal(rs[:], rs[:])
        out = pool.tile([P, DA], F32, tag="ln_o", name="ln_o", bufs=1)
        nc.vector.scalar_tensor_tensor(out[:], xc[:], rs[:], g_rep[:],
                                       op0=OP.mult, op1=OP.mult)
        nc.vector.tensor_tensor(out[:], out[:], b_rep[:], op=OP.add)
        return out

    def pair_allreduce(sb_tiles, nm):
        """AllReduce 4 x [128, DA] tiles across the core pair; returns dram tile."""
        ar_in = dramp.tile([T, DA], F32, name=nm + "_in")
        ar_out = dramp.tile([T, DA], F32, name=nm + "_out")
        for t in range(NT):
            nc.sync.dma_start(ar_in[t * P:(t + 1) * P, :], sb_tiles[t][:])
        nc.gpsimd.collective_compute(
            "AllReduce", OP.add, replica_groups=PAIRS,
            ins=[ar_in.opt()], outs=[ar_out.opt()],
        )
        return ar_out

    def block(actp, wp, x_tiles, W, pfxs, xtags):
        pfx = pfxs[0]
        """x_tiles: 4 x [128, DA] f32 token-major. Pair-sharded heads + DFF."""
        wqkv_sb = []
        for dt_ in range(4):
            t_ = wp.tile([P, 3 * DH], F32R, tag=f"wqkv{dt_}", name=f"{pfx}wqkv{dt_}")
            nc.sync.dma_start(t_[:], W["wqkv"][dt_ * P:(dt_ + 1) * P, :])
            wqkv_sb.append(t_)
        wo_sb = []
        for dt_ in range(2):
            t_ = wp.tile([P, DA], F32R, tag=f"wo{dt_}", name=f"{pfx}wo{dt_}")
            nc.sync.dma_start(t_[:], W["wo"][dt_ * P:(dt_ + 1) * P, :])
            wo_sb.append(t_)
        w1_sb = []
        for dt_ in range(4):
            t_ = wp.tile([P, FLOC], F32R, tag=f"w1{dt_}", name=f"{pfx}w1{dt_}")
            nc.sync.dma_start(t_[:], W["w1"][dt_ * P:(dt_ + 1) * P, :])
            w1_sb.append(t_)
        w2_sb = []
        for ft in range(8):
            t_ = wp.tile([P, DA], F32R, tag=f"w2{ft}", name=f"{pfx}w2{ft}")
            nc.sync.dma_start(t_[:], W["w2"][ft * P:(ft + 1) * P, :])
            w2_sb.append(t_)

        # ---- ln1 + qkv (own heads: q|k|v each DH=256 cols) ----
        qkv = []
        for t in range(NT):
            ln = layernorm(actp, x_tiles[t][:], rep_sb[pfx + "_ln1g"],
                           rep_sb[pfx + "_ln1b"], pfx + "ln1")
            lnT = [transpose_to(actp, ln[:, dt_ * P:(dt_ + 1) * P], P, P, "lnT")
                   for dt_ in range(4)]
            qkv_t = actp.tile([P, 3 * DH], F32R, tag=f"qkv{t}", name=f"{pfx}qkv{t}")
            for j0, j1 in [(0, 512), (512, 768)]:
                acc = psA.tile([P, j1 - j0], F32, tag="acc", name="acc")
                for dt_ in range(4):
                    nc.tensor.matmul(acc[:], lnT[dt_][:], wqkv_sb[dt_][:, j0:j1],
                                     start=(dt_ == 0), stop=(dt_ == 3))
                nc.vector.tensor_copy(qkv_t[:, j0:j1], acc[:])
            qkv.append(qkv_t)

        # ---- attention (4 local heads) ----
        qT, kT = [], []
        for dt_ in range(2):
            qTt = actp.tile([P, T], F32R, tag=f"qT{dt_}", name=f"{pfx}qT{dt_}")
            kTt = actp.tile([P, T], F32R, tag=f"kT{dt_}", name=f"{pfx}kT{dt_}")
            for t in range(NT):
                pt = psT.tile([P, P], F32, tag="ptp", name="ptp")
                nc.tensor.transpose(pt[:], qkv[t][:, dt_ * P:(dt_ + 1) * P].bitcast(F32),
                                    ident[:])
                nc.vector.tensor_scalar_mul(qTt[:, t * P:(t + 1) * P], pt[:],
                                            1.0 / float(np.sqrt(HD)))
                pt2 = psT.tile([P, P], F32, tag="ptp", name="ptp")
                nc.tensor.transpose(pt2[:],
                                    qkv[t][:, DH + dt_ * P:DH + (dt_ + 1) * P].bitcast(F32),
                                    ident[:])
                nc.vector.tensor_copy(kTt[:, t * P:(t + 1) * P], pt2[:])
            qT.append(qTt)
            kT.append(kTt)

        o_sb = [actp.tile([P, DH], F32R, tag=f"o{t}", name=f"{pfx}o{t}")
                for t in range(NT)]
        for h in range(HLOC):
            dt_, half = h // 2, h % 2
            qh = qT[dt_][half * HD:(half + 1) * HD, :]
            kh = kT[dt_][half * HD:(half + 1) * HD, :]
            for t in range(NT):
                sc = psA.tile([P, T], F32, tag="acc", name="acc")
                nc.tensor.matmul(sc[:], qh[:, t * P:(t + 1) * P], kh[:],
                                 start=True, stop=True)
                pex = actp.tile([P, T], F32, tag="pex", name="pex", bufs=2)
                rsum = actp.tile([P, 1], F32, tag="rsum", name="rsum", bufs=2)
                nc.vector.tensor_tensor(pex[:], sc[:],
                                        mask_sb[:, t * T:(t + 1) * T], op=OP.add)
                nc.scalar.activation(pex[:], pex[:], AF.Exp, accum_out=rsum[:])
                oacc = psS.tile([P, HD], F32, tag="sm", name="sm")
                for tk in range(t + 1):
                    pTt = psT.tile([P, P], F32, tag="ptp", name="ptp")
                    nc.tensor.transpose(pTt[:], pex[:, tk * P:(tk + 1) * P], ident[:])
                    pTs = actp.tile([P, P], F32R, tag="pTs", name="pTs", bufs=2)
                    nc.vector.tensor_copy(pTs[:], pTt[:])
                    nc.tensor.matmul(oacc[:], pTs[:],
                                     qkv[tk][:, 2 * DH + h * HD:2 * DH + (h + 1) * HD],
                                     start=(tk == 0), stop=(tk == t))
                rr = actp.tile([P, 1], F32, tag="rr", name="rr", bufs=3)
                nc.vector.reciprocal(rr[:], rsum[:])
                nc.vector.tensor_scalar(o_sb[t][:, h * HD:(h + 1) * HD], oacc[:],
                                        rr[:], None, op0=OP.mult)

        # ---- partial o @ wo -> pair AllReduce -> residual ----
        attn_p = []
        for t in range(NT):
            oT = [transpose_to(actp, o_sb[t][:, dt_ * P:(dt_ + 1) * P], P, P, "oT", bufs=3)
                  for dt_ in range(2)]
            acc = psA.tile([P, DA], F32, tag="acc", name="acc")
            for dt_ in range(2):
                nc.tensor.matmul(acc[:], oT[dt_][:], wo_sb[dt_][:],
                                 start=(dt_ == 0), stop=(dt_ == 1))
            ap_t = actp.tile([P, DA], F32, tag="prt", name="attn_p", bufs=2)
            nc.vector.tensor_copy(ap_t[:], acc[:])
            attn_p.append(ap_t)
        ar_attn = pair_allreduce(attn_p, pfxs + "attn")
        x2 = []
        for t in range(NT):
            ld = actp.tile([P, DA], F32, tag="s512", name="ar_ld", bufs=3)
            nc.sync.dma_start(ld[:], ar_attn[t * P:(t + 1) * P, :])
            x2t = actp.tile([P, DA], F32, tag=f"x2_{t}", name=f"{pfx}x2_{t}")
            nc.vector.tensor_tensor(x2t[:], x_tiles[t][:], ld[:], op=OP.add)
            x2.append(x2t)

        # ---- ln2 + MLP (own DFF half) -> pair AllReduce -> residual ----
        mlp_p = []
        for t in range(NT):
            ln = layernorm(actp, x2[t][:], rep_sb[pfx + "_ln2g"],
                           rep_sb[pfx + "_ln2b"], pfx + "ln2")
            lnT = [transpose_to(actp, ln[:, dt_ * P:(dt_ + 1) * P], P, P, "lnT")
                   for dt_ in range(4)]
            h1g = actp.tile([P, FLOC], F32R, tag="h1g", name="h1g", bufs=1)
            for j in range(2):
                acc = psA.tile([P, DA], F32, tag="acc", name="acc")
                for dt_ in range(4):
                    nc.tensor.matmul(acc[:], lnT[dt_][:],
                                     w1_sb[dt_][:, j * DA:(j + 1) * DA],
                                     start=(dt_ == 0), stop=(dt_ == 3))
                nc.scalar.activation(h1g[:, j * DA:(j + 1) * DA], acc[:],
                                     AF.Gelu_apprx_tanh)
            acc2 = psA.tile([P, DA], F32, tag="acc", name="acc")
            for ft in range(8):
                hT = transpose_to(actp, h1g[:, ft * P:(ft + 1) * P], P, P, "hT",
                                  bufs=3)
                nc.tensor.matmul(acc2[:], hT[:], w2_sb[ft][:],
                                 start=(ft == 0), stop=(ft == 7))
            mp_t = actp.tile([P, DA], F32, tag="prt", name="mlp_p", bufs=2)
            nc.vector.tensor_copy(mp_t[:], acc2[:])
            mlp_p.append(mp_t)
        ar_mlp = pair_allreduce(mlp_p, pfxs + "mlp")
        x3 = []
        for t in range(NT):
            ld = actp.tile([P, DA], F32, tag="s512", name="ar_ld2", bufs=3)
            nc.sync.dma_start(ld[:], ar_mlp[t * P:(t + 1) * P, :])
            x3t = actp.tile([P, DA], F32, tag=xtags[t], name=f"{pfx}x3_{t}")
            nc.vector.tensor_tensor(x3t[:], x2[t][:], ld[:], op=OP.add)
            x3.append(x3t)
        return x3

    for rep in range(REPS):
        sfx = f"_r{rep}"
        hAp = openpool("hAp" + sfx, bufs=1)
        actA = openpool("actA" + sfx, bufs=1)
        wA = openpool("wA" + sfx, bufs=1)

        # ===== embed =====
        x0 = []
        for t in range(NT):
            xa = actA.tile([P, DA], F32, tag=f"x0_{t}", name=f"x0_{t}")
            nc.gpsimd.indirect_dma_start(
                out=xa[:], out_offset=None, in_=emb[:],
                in_offset=bass.IndirectOffsetOnAxis(ap=ids_sb[:, t:t + 1], axis=0),
            )
            pe = actA.tile([P, DA], F32, tag="s512", name=f"pe_{t}", bufs=3)
            nc.sync.dma_start(pe[:], posenc[t * P:(t + 1) * P, :])
            nc.vector.tensor_tensor(xa[:], xa[:], pe[:], op=OP.add)
            x0.append(xa)

        # ===== block A (hA tiles into hAp pool) =====
        hA_raw = block(actA, wA, x0, blkw["a"], "a" + sfx,
                       [f"x0_{t}" for t in range(NT)])
        hA = []
        for t in range(NT):
            h_ = hAp.tile([P, DA], F32, tag=f"hA_{t}", name=f"hA_{t}")
            nc.vector.tensor_copy(h_[:], hA_raw[t][:])
            hA.append(h_)
        closepool(wA)
        closepool(actA)

        # ===== z + allgather =====
        mp = openpool("mp" + sfx, bufs=1)
        zp = psS.tile([1, DA], F32, tag="sm", name="zp")
        for t in range(NT):
            nc.tensor.matmul(zp[:], ones_col[:], hA[t][:], start=(t == 0),
                             stop=(t == NT - 1))
        z_sb = mp.tile([1, DA], F32, tag="z_sb", name="z_sb")
        nc.vector.tensor_scalar_mul(z_sb[:], zp[:], 1.0 / T)
        zb_in = dramp.tile([1, DA], F32, name="zb_in" + sfx)
        zb_out = dramp.tile([B, DA], F32, name="zb_out" + sfx)
        nc.sync.dma_start(zb_in[:], z_sb[:])
        nc.gpsimd.collective_compute(
            "AllGather", OP.bypass, replica_groups=BGRPS,
            ins=[zb_in.opt()], outs=[zb_out.opt()],
        )
        zall = mp.tile([B, DA], F32, tag="zall", name="zall")
        nc.sync.dma_start(zall[:], zb_out[:])

        # ===== M = sum_a wq_a wk_a^T / (A sqrt(DK)); combqT =====
        wq_sb, wk_sb = [], []
        for a in range(A):
            tq = mp.tile([DK, DA], F32R, tag=f"wq{a}", name=f"wq{a}")
            nc.sync.dma_start(tq[:], wqT[a])
            wq_sb.append(tq)
            tk_ = mp.tile([DK, D], F32R, tag=f"wk{a}", name=f"wk{a}")
            nc.sync.dma_start(tk_[:], wkT[a])
            wk_sb.append(tk_)
        M_sb = []
        for dt_ in range(4):
            mt = mp.tile([P, D], F32R, tag=f"M{dt_}", name=f"M{dt_}")
            for ec in range(4):
                acc = psA.tile([P, DA], F32, tag="acc", name="acc")
                for a in range(A):
                    nc.tensor.matmul(acc[:], wq_sb[a][:, dt_ * P:(dt_ + 1) * P],
                                     wk_sb[a][:, ec * DA:(ec + 1) * DA],
                                     start=(a == 0), stop=(a == A - 1))
                nc.vector.tensor_scalar_mul(mt[:, ec * DA:(ec + 1) * DA], acc[:],
                                            1.0 / (A * float(np.sqrt(DK))))
            M_sb.append(mt)
        zT = []
        for dt_ in range(4):
            pt = psT.tile([P, P], F32, tag="ptp", name="ptp")
            nc.tensor.transpose(pt[:P, :B], zall[:, dt_ * P:(dt_ + 1) * P],
                                ident[:B, :B])
            zt = mp.tile([P, B], F32R, tag=f"zT{dt_}", name=f"zT{dt_}")
            nc.vector.tensor_copy(zt[:], pt[:P, :B])
            zT.append(zt)
        cqT = mp.tile([P, 16 * B], F32, tag="cqT", name="cqT")
        for et in range(16):
            acc = psS.tile([P, B], F32, tag="sm", name="sm")
            for dt_ in range(4):
                nc.tensor.matmul(acc[:], M_sb[dt_][:, et * P:(et + 1) * P], zT[dt_][:],
                                 start=(dt_ == 0), stop=(dt_ == 3))
            nc.vector.tensor_copy(cqT[:, et * B:(et + 1) * B], acc[:])

        # ===== pool scan (fp32, DMA-bound) =====
        scanp = openpool("scan" + sfx, bufs=12)
        comb_loc = mp.tile([B, NSH], F32, tag="comb_loc", name="comb_loc")
        for nch in range(NSH // DA):
            acc = psS.tile([B, DA], F32, tag="sm", name="sm")
            for dt_ in range(16):
                ptile = scanp.tile([P, DA], F32, tag="ptile", name="ptile")
                nc.sync.dma_start(
                    ptile[:], poolT_sh[dt_ * P:(dt_ + 1) * P, nch * DA:(nch + 1) * DA])
                nc.tensor.matmul(acc[:], cqT[:, dt_ * B:(dt_ + 1) * B], ptile[:],
                                 start=(dt_ == 0), stop=(dt_ == 15))
            nc.vector.tensor_copy(comb_loc[:, nch * DA:(nch + 1) * DA], acc[:])
        closepool(scanp)
        cb_in = dramp.tile([B, NSH], F32, name="cb_in" + sfx)
        cb_out = dramp.tile([NCORES * B, NSH], F32, name="cb_out" + sfx)
        nc.sync.dma_start(cb_in[:], comb_loc[:])
        nc.gpsimd.collective_compute(
            "AllGather", OP.bypass, replica_groups=ALLG,
            ins=[cb_in.opt()], outs=[cb_out.opt()],
        )
        closepool(mp)

        # ===== top-16 =====
        tkp = openpool("tkp" + sfx, bufs=1)
        comb32 = tkp.tile([32, NSH], F32, tag="comb32", name="comb32")
        nc.sync.dma_start(comb32[:], cb_out[:])
        v1 = tkp.tile([32, 8], F32, tag="v1", name="v1")
        i1 = tkp.tile([32, 8], U32, tag="i1", name="i1")
        v2 = tkp.tile([32, 8], F32, tag="v2", name="v2")
        i2 = tkp.tile([32, 8], U32, tag="i2", name="i2")
        scr = tkp.tile([32, NSH], F32, tag="scr", name="scr")
        nc.vector.max(v1[:], comb32[:])
        nc.vector.max_index(i1[:], v1[:], comb32[:])
        nc.vector.match_replace(scr[:], v1[:], comb32[:], -1e30)
        nc.vector.max(v2[:], scr[:])
        nc.vector.max_index(i2[:], v2[:], comb32[:])
        if32 = tkp.tile([32, 16], F32, tag="if32", name="if32")
        nc.vector.tensor_copy(if32[:, 0:8], i1[:])
        nc.vector.tensor_copy(if32[:, 8:16], i2[:])
        nc.vector.tensor_scalar(if32[:], if32[:], small["segoff"][:32, :], None,
                                op0=OP.add)
        v12 = tkp.tile([32, 16], F32, tag="v12", name="v12")
        nc.vector.tensor_copy(v12[:, 0:8], v1[:])
        nc.vector.tensor_copy(v12[:, 8:16], v2[:])

        # regroup to per-b rows [4, 128] via DRAM bounce
        vb_d = dramp.tile([32, 16], F32, name="vb_d" + sfx)
        ib_d = dramp.tile([32, 16], F32, name="ib_d" + sfx)
        nc.sync.dma_start(vb_d[:], v12[:])
        nc.sync.dma_start(ib_d[:], if32[:])
        vals2 = tkp.tile([B, 128], F32, tag="vals2", name="vals2")
        idx2 = tkp.tile([B, 128], F32, tag="idx2", name="idx2")
        nc.sync.dma_start(vals2[:].rearrange("b (c k) -> b c k", c=8),
                          vb_d[:].rearrange("(c b) k -> b c k", b=B))
        nc.sync.dma_start(idx2[:].rearrange("b (c k) -> b c k", c=8),
                          ib_d[:].rearrange("(c b) k -> b c k", b=B))

        tv1 = tkp.tile([B, 8], F32, tag="tv1", name="tv1")
        tv2 = tkp.tile([B, 8], F32, tag="tv2", name="tv2")
        scr2 = tkp.tile([B, 128], F32, tag="scr2", name="scr2")
        nc.vector.max(tv1[:], vals2[:])
        nc.vector.match_replace(scr2[:], tv1[:], vals2[:], -1e30)
        nc.vector.max(tv2[:], scr2[:])
        topv = tkp.tile([B, 16], F32, tag="topv", name="topv")
        nc.vector.tensor_copy(topv[:, 0:8], tv1[:])
        nc.vector.tensor_copy(topv[:, 8:16], tv2[:])

        # global indices by equality match
        cmp_t = tkp.tile([B, 16 * 128], F32, tag="cmp_t", name="cmp_t")
        cmp_v = cmp_t[:].rearrange("b (k n) -> b k n", k=16)
        va = vals2[:].unsqueeze(1).broadcast_to([B, 16, 128])
        ta = topv[:].unsqueeze(2).broadcast_to([B, 16, 128])
        ia = idx2[:].unsqueeze(1).broadcast_to([B, 16, 128])
        nc.vector.tensor_tensor(cmp_v, va, ta, op=OP.is_equal)
        nc.vector.tensor_tensor(cmp_v, cmp_v, ia, op=OP.mult)
        gidf = tkp.tile([B, 16], F32, tag="gidf", name="gidf")
        nc.vector.tensor_reduce(gidf[:], cmp_v, axis=AX.X, op=OP.add)

        # ===== alphas =====
        th = tkp.tile([B, 1], F32, tag="th", name="th")
        nc.vector.tensor_reduce(th[:], topv[:], axis=AX.X, op=OP.min)
        nth = tkp.tile([B, 1], F32, tag="nth", name="nth")
        nc.vector.tensor_scalar_mul(nth[:], th[:], -1.0)
        es = tkp.tile([B, 16], F32, tag="es", name="es")
        ss = tkp.tile([B, 1], F32, tag="ss", name="ss")
        nc.scalar.activation(es[:], topv[:], AF.Exp, bias=nth[:], accum_out=ss[:])
        rlam = tkp.tile([B, 1], F32, tag="rlam", name="rlam")
        nc.vector.reciprocal(rlam[:], small["lam"][:])
        nthl = tkp.tile([B, 1], F32, tag="nthl", name="nthl")
        nc.vector.tensor_tensor(nthl[:], nth[:], rlam[:], op=OP.mult)
        eh = tkp.tile([B, 16], F32, tag="eh", name="eh")
        sh = tkp.tile([B, 1], F32, tag="sh", name="sh")
        nc.scalar.activation(eh[:], topv[:], AF.Exp, bias=nthl[:], scale=rlam[:],
                             accum_out=sh[:])
        nc.vector.reciprocal(ss[:], ss[:])
        nc.vector.reciprocal(sh[:], sh[:])
        als = tkp.tile([B, 16], F32, tag="als", name="als")
        alh = tkp.tile([B, 16], F32, tag="alh", name="alh")
        nc.vector.tensor_scalar(als[:], es[:], ss[:], None, op0=OP.mult)
        nc.vector.tensor_scalar(alh[:], eh[:], sh[:], None, op0=OP.mult)
        al = tkp.tile([B, 16], F32, tag="al", name="al")
        nc.vector.tensor_tensor(al[:], als[:], alh[:], op=OP.subtract)
        nc.vector.tensor_scalar(al[:], al[:], small["warm"][:], None, op0=OP.mult)
        nc.vector.tensor_tensor(al[:], al[:], alh[:], op=OP.add)

        # ===== local gather + partial mix + AllReduce =====
        lidf = tkp.tile([B, 16], F32, tag="lidf", name="lidf")
        nc.vector.tensor_scalar(lidf[:], gidf[:], small["coff"][:], None,
                                op0=OP.subtract)
        neg = tkp.tile([B, 16], F32, tag="neg", name="neg")
        nc.vector.tensor_scalar(neg[:], lidf[:], 0.0, None, op0=OP.is_lt)
        nc.vector.scalar_tensor_tensor(lidf[:], neg[:], 70000.0, lidf[:],
                                       op0=OP.mult, op1=OP.add)
        lid_d = dramp.tile([B, KMAX], F32, name="lid_d" + sfx)
        nc.sync.dma_start(lid_d[:], lidf[:])
        lid64 = tkp.tile([B * KMAX, 1], F32, tag="lid64", name="lid64")
        nc.sync.dma_start(lid64[:], lid_d[:].rearrange("b k -> (b k)").unsqueeze(1))
        lid64u = tkp.tile([B * KMAX, 1], U32, tag="lid64u", name="lid64u")
        nc.vector.tensor_copy(lid64u[:], lid64[:])
        g64 = tkp.tile([B * KMAX, D], F32, tag="g64", name="g64")
        nc.vector.memset(g64[:], 0.0)
        nc.gpsimd.indirect_dma_start(
            out=g64[:], out_offset=None, in_=pool_sh[:],
            in_offset=bass.IndirectOffsetOnAxis(ap=lid64u[:, 0:1], axis=0),
            bounds_check=NSH - 1, oob_is_err=False,
        )
        al_d = dramp.tile([B, KMAX], F32, name="al_d" + sfx)
        nc.sync.dma_start(al_d[:], al[:])
        al64 = tkp.tile([B * KMAX, 1], F32, tag="al64", name="al64")
        nc.sync.dma_start(al64[:], al_d[:].rearrange("b k -> (b k)").unsqueeze(1))
        albd = tkp.tile([B * KMAX, B], F32, tag="albd", name="albd")
        nc.vector.memset(albd[:], 0.0)
        for b_ in range(B):
            nc.sync.dma_start(albd[b_ * KMAX:(b_ + 1) * KMAX, b_:b_ + 1],
                              al64[b_ * KMAX:(b_ + 1) * KMAX, :])
        mixed_p = tkp.tile([B, D], F32, tag="mixed_p", name="mixed_p")
        for ch in range(4):
            acc = psS.tile([B, DA], F32, tag="sm", name="sm")
            nc.tensor.matmul(acc[:], albd[:], g64[:, ch * DA:(ch + 1) * DA],
                             start=True, stop=True)
            nc.vector.tensor_copy(mixed_p[:, ch * DA:(ch + 1) * DA], acc[:])
        mx_in = dramp.tile([B, D], F32, name="mx_in" + sfx)
        mx_out = dramp.tile([B, D], F32, name="mx_out" + sfx)
        nc.sync.dma_start(mx_in[:], mixed_p[:])
        nc.gpsimd.collective_compute(
            "AllReduce", OP.add, replica_groups=ALLG,
            ins=[mx_in.opt()], outs=[mx_out.opt()],
        )
        mixed = tkp.tile([B, D], F32, tag="mixed", name="mixed")
        nc.sync.dma_start(mixed[:], mx_out[:])

        # ===== W_b = w_base + gam * U V  (own b via one-hot) =====
        mixb = tkp.tile([1, D], F32, tag="mixb", name="mixb")
        for ch in range(4):
            acc = psS.tile([B, DA], F32, tag="sm", name="sm")
            nc.tensor.matmul(acc[:1, :], small["eb"][:], mixed[:, ch * DA:(ch + 1) * DA],
                             start=True, stop=True)
            nc.vector.tensor_copy(mixb[:, ch * DA:(ch + 1) * DA], acc[:1, :])
        mb_d = dramp.tile([1, D], F32, name="mb_d" + sfx)
        nc.sync.dma_start(mb_d[:], mixb[:])
        UT = tkp.tile([R, DA], F32, tag="UT", name="UT")
        nc.sync.dma_start(UT[:], mb_d[0:1, 0:DA * R].rearrange("a (d r) -> (a r) d", r=R))
        Vm = tkp.tile([R, DB], F32, tag="Vm", name="Vm")
        nc.sync.dma_start(Vm[:], mb_d[0:1, DA * R:].rearrange("a (r o) -> (a r) o", r=R))
        W_sb = []
        for dt_ in range(4):
            acc = psA.tile([P, DB], F32, tag="acc", name="acc")
            nc.tensor.matmul(acc[:], UT[:, dt_ * P:(dt_ + 1) * P], Vm[:],
                             start=True, stop=True)
            wt_ = hAp.tile([P, DB], F32R, tag=f"W{dt_}", name=f"W{dt_}")
            nc.vector.scalar_tensor_tensor(wt_[:], acc[:], small["gam"][:], wb_sb[dt_][:],
                                           op0=OP.mult, op1=OP.add)
            W_sb.append(wt_)

        closepool(tkp)

        # ===== h_mid = LN(hA @ W + b_base) =====
        actB = openpool("actB" + sfx, bufs=1)
        hmid = []
        for t in range(NT):
            hT = [transpose_to(actB, hA[t][:, dt_ * P:(dt_ + 1) * P], P, P, "lnT")
                  for dt_ in range(4)]
            acc = psA.tile([P, DB], F32, tag="acc", name="acc")
            for dt_ in range(4):
                nc.tensor.matmul(acc[:], hT[dt_][:], W_sb[dt_][:],
                                 start=(dt_ == 0), stop=(dt_ == 3))
            pre = actB.tile([P, DB], F32, tag="s512", name=f"pre{t}", bufs=3)
            nc.vector.tensor_tensor(pre[:], acc[:], rep_sb["b_base"][:], op=OP.add)
            hm = layernorm(actB, pre[:], rep_sb["mid_g"], rep_sb["mid_b"], "mid")
            hmc = actB.tile([P, DB], F32, tag=f"hm_{t}", name=f"hm_{t}")
            nc.vector.tensor_copy(hmc[:], hm[:])
            hmid.append(hmc)

        # ===== block B =====
        wB = openpool("wB" + sfx, bufs=1)
        hout = block(actB, wB, hmid, blkw["b"], "b" + sfx,
                     [f"hm_{t}" for t in range(NT)])
        closepool(wB)

        # ===== h_out^T allgather (b-groups) =====
        ht_in = dramp.tile([DB, T], F32R, name="ht_in" + sfx)
        ht_out = dramp.tile([B * DB, T], F32R, name="ht_out" + sfx)
        for dt_ in range(4):
            htile = actB.tile([P, T], F32R, tag="houtT", name="houtT", bufs=1)
            for t in range(NT):
                pt = psT.tile([P, P], F32, tag="ptp", name="ptp")
                nc.tensor.transpose(pt[:], hout[t][:, dt_ * P:(dt_ + 1) * P], ident[:])
                nc.vector.tensor_copy(htile[:, t * P:(t + 1) * P], pt[:])
            nc.sync.dma_start(ht_in[dt_ * P:(dt_ + 1) * P, :], htile[:])
        nc.gpsimd.collective_compute(
            "AllGather", OP.bypass, replica_groups=BGRPS,
            ins=[ht_in.opt()], outs=[ht_out.opt()],
        )
        closepool(actB)
        closepool(hAp)

        # ===== lm_head (vocab shard) =====
        lmp = openpool("lmp" + sfx, bufs=1)
        lms = openpool("lms" + sfx, bufs=6)
        hoT = []
        for i in range(16):
            t_ = lmp.tile([P, T], F32R, tag=f"hoT{i}", name=f"hoT{i}")
            nc.sync.dma_start(t_[:], ht_out[i * P:(i + 1) * P, :])
            hoT.append(t_)
        NVC = 8
        VC = VSH // NVC  # 500
        for b_ in range(B):
            for t in range(NT):
                for vc in range(NVC):
                    lw = []
                    for dt_ in range(4):
                        lw_t = lms.tile([P, VC], F32R, tag="lw", name="lw")
                        nc.sync.dma_start(
                            lw_t[:], lmh_sh[dt_ * P:(dt_ + 1) * P, vc * VC:(vc + 1) * VC])
                        lw.append(lw_t)
                    acc = psA.tile([P, VC], F32, tag="acc", name="acc")
                    for dt_ in range(4):
                        nc.tensor.matmul(acc[:], hoT[b_ * 4 + dt_][:, t * P:(t + 1) * P],
                                         lw[dt_][:], start=(dt_ == 0), stop=(dt_ == 3))
                    ot = lms.tile([P, VC], F32, tag="ot", name="ot")
                    nc.vector.tensor_copy(ot[:], acc[:])
                    nc.sync.dma_start(
                        logits[b_ * T + t * P:b_ * T + (t + 1) * P,
                               vc * VC:(vc + 1) * VC], ot[:])
        closepool(lms)
        closepool(lmp)

    for p in [dramp, psS, psT, psA, cst]:
        closepool(p)


def build(REPS=1):
    nc = bacc.Bacc(None, target_bir_lowering=False, num_devices=NCORES)
    with tile.TileContext(nc) as tc:
        _bld(nc, tc, REPS=REPS)
    nc.compile()
    return nc


# ---------------------------------------------------------------- host side

def _np_posenc():
    pos = np.arange(T, dtype=np.float32)[:, None]
    i = np.arange(DA // 2, dtype=np.float32)[None, :]
    ang = pos / np.power(np.float32(10000.0), 2 * i / DA, dtype=np.float32)
    return np.concatenate([np.sin(ang), np.cos(ang)], axis=-1)[:, :DA].astype(np.float32)


def make_in_maps(inputs):
    f32 = lambda x: np.asarray(x, dtype=np.float32)
    rep = lambda v: np.ascontiguousarray(np.tile(f32(v)[None, :], (128, 1)))
    posenc = _np_posenc()
    tril = np.tril(np.ones((T, T), dtype=bool))
    madd = np.where(tril, np.float32(0.0), np.float32(-1e9)).astype(np.float32)
    maskre = np.ascontiguousarray(
        madd.reshape(NT, 128, T).transpose(1, 0, 2).reshape(128, NT * T))
    ident = np.eye(128, dtype=np.float32)
    segoff = ((np.arange(32) // B) * NSH).astype(np.float32).reshape(32, 1)
    pool = f32(inputs["pool"])
    lam = float(np.asarray(inputs["lambda_val"]))
    warm = float(bool(np.asarray(inputs["is_warmup"])))
    gam = float(np.asarray(inputs["gamma"]))
    wq, wk = f32(inputs["wq"]), f32(inputs["wk"])
    lm_head = f32(inputs["lm_head"])
    shared = {
        "emb": f32(inputs["emb"]),
        "posenc": posenc, "maskre": maskre, "ident": ident, "segoff": segoff,
        "wqT": np.ascontiguousarray(wq.transpose(0, 2, 1)),
        "wkT": np.ascontiguousarray(wk.transpose(0, 2, 1)),
        "w_base": f32(inputs["w_base"]),
        "lam": np.full((B, 1), lam, np.float32),
        "warm": np.full((B, 1), warm, np.float32),
        "gam": np.full((128, 1), gam, np.float32),
        "a_ln1g_rep": rep(inputs["a_ln1g"]), "a_ln1b_rep": rep(inputs["a_ln1b"]),
        "a_ln2g_rep": rep(inputs["a_ln2g"]), "a_ln2b_rep": rep(inputs["a_ln2b"]),
        "b_ln1g_rep": rep(inputs["b_ln1g"]), "b_ln1b_rep": rep(inputs["b_ln1b"]),
        "b_ln2g_rep": rep(inputs["b_ln2g"]), "b_ln2b_rep": rep(inputs["b_ln2b"]),
        "mid_g_rep": rep(inputs["mid_ln_g"]), "mid_b_rep": rep(inputs["mid_ln_b"]),
        "b_base_rep": rep(inputs["b_base"]),
    }
    ids_all = np.asarray(inputs["input_ids"]).astype(np.uint32)
    in_maps = []
    for c in range(NCORES):
        b, s = c // 2, c % 2
        m = dict(shared)
        m["ids"] = np.ascontiguousarray(ids_all[b].reshape(NT, 128).T)
        m["pool_sh"] = np.ascontiguousarray(pool[c * NSH:(c + 1) * NSH, :])
        m["poolT_sh"] = np.ascontiguousarray(pool[c * NSH:(c + 1) * NSH, :].T)
        m["lmh_sh"] = np.ascontiguousarray(lm_head[:, c * VSH:(c + 1) * VSH])
        m["eb"] = np.eye(B, dtype=np.float32)[:, b:b + 1].copy()
        m["coff"] = np.full((B, 1), float(c * NSH), np.float32)
        for pfx in ["a", "b"]:
            wqkv = f32(inputs[pfx + "_wqkv"])
            cols = [wqkv[:, j * DA + s * DH:(j * DA) + (s + 1) * DH] for j in range(3)]
            m[pfx + "_wqkv_sh"] = np.ascontiguousarray(np.concatenate(cols, axis=1))
            m[pfx + "_wo_sh"] = np.ascontiguousarray(
                f32(inputs[pfx + "_wo"])[s * DH:(s + 1) * DH, :])
            m[pfx + "_w1_sh"] = np.ascontiguousarray(
                f32(inputs[pfx + "_w1"])[:, s * FLOC:(s + 1) * FLOC])
            m[pfx + "_w2_sh"] = np.ascontiguousarray(
                f32(inputs[pfx + "_w2"])[s * FLOC:(s + 1) * FLOC, :])
        in_maps.append(m)
    return in_maps


def assemble(results):
    return np.concatenate(
        [results[c]["logits"].reshape(B, T, VSH) for c in range(NCORES)], axis=-1)


_NC_CACHE = {}


def kernel(**inputs) -> np.ndarray:
    if "nc" not in _NC_CACHE:
        _NC_CACHE["nc"] = build()
    nc = _NC_CACHE["nc"]
    in_maps = make_in_maps(inputs)
    res = run_bass_kernel_spmd(nc, in_maps, core_ids=list(range(NCORES)))
    return assemble(res.results)
